# revision 41
# baseline (speedup 1.0000x reference)
"""Trainium2 Bass kernel for nn_AttentionLayer (sparse_attention).

Reference computation (B=4, N=2048, C=256, H=8, HD=32):
    qkv = x @ qkv_w.T; q,k,v = split(qkv); heads
    scores = q k^T / sqrt(HD) + adj          [B,H,N,N]
    out    = softmax(scores) @ v             -> merge heads [B,N,C]
    result = (out*0.1 + x) @ out_w.T + out_b
(The pos_proj(adj) value in the reference is dead code; x0 is unused.)

Sharding: 8 cores = (batch b, query-half).  Core c handles batch c//2 and
query rows [(c%2)*1024, (c%2+1)*1024).  Each core computes K/V for its
whole batch locally (no collectives); the host rolls the key axis so the
core's own query rows are rows 0..1023 of its x input, and rolls adj
columns the same way (softmax is key-permutation invariant).

fp8 / DoubleRow design (measured on HW, not just the cost model):
  * All attention-core matmuls run on fp8e4 operands.  Measured per-call
    costs on TRN2 (512 moving cols): 128-row stationary fp8 ~175ns,
    fp8-DoubleRow ~190ns, but 32/64-row stationary tiles are 2.3x SLOWER
    (~374-438ns) because of PE row-tiling, and LdWeights (unmodeled in
    the cost model) makes stationary-free-256 DR calls load-bound.
  * scores: per-head zero-padded k tiles [P, N] (head rows at 32*(h%4),
    rest zeroed once at build time) contract a full 128-row stationary
    tile against packed q stacks [P, NQ] -> plain fp8 matmuls at the
    fast 128-row rate.  The q rows of other heads hit k's zero rows.
  * adj is pre-added into PSUM by the PE: adj*16 splits on the host into
    c (multiples of 8, exact in fp8) + r (|r|<=4), shipped interleaved
    [P,16,2chunk,2slot,512], accumulated via a DoubleRow identity
    matmul.  This removes ALL per-tile DVE/Pool combine work (the old
    kernel spent ~110us of DVE on adj adds).
  * ACT exps PSUM scores straight to fp8 E tiles (scale=1/16 undoes the
    q/k fp8 scaling; bias=-2ln2 rescales E by 1/4 so exp(max) stays
    under the fp8e4 max of 240 — the softmax ratio is invariant).  ACT
    is the bottleneck engine (~134us of exp stream); the schedule keeps
    it >95% dense.
  * attention@V runs DoubleRow over KEY-TILE PAIRS: E tiles are
    [P, 2chunk, 2slot(kt-parity), 512] fp8 and v is fp8 [P,16,H,64]
    (64-col stationary: DR outputs must sit at tile col 0, so heads are
    processed singly with po rows 0..63).  attnv emission lags the
    exp stream by 2 kt-pairs so the in-order PE never stalls ACT.
  * QKV projections are fp8 DoubleRow over host-prepped [P,2,*] x/w
    (channel c = 128*slot + p, one call contracts all 256 channels; x16
    weight pre-scale keeps fp8 weights normal, divided out in the
    PSUM->fp8 copies).  Group 0's copies run on the pre-stream-idle ACT
    so the exp stream starts ~6us in; v and groups 1-3 hide under it.
  * out = E @ [v | 10]: po row 32 gives 10*sum(E) (softmax denominator
    with the 0.1 output scale folded).  Reciprocal runs on a DMA-spread
    [32,32] layout; residual adds fold into each attT chunk's last head
    so the tail is only norm(h7) + out_proj + output DMAs.
  * Hybrid exp: ~30% of the exp tiles run on DVE via a runtime-registered
    custom DVE op (cubic Horner exp(s) * host-precomputed exp(adj)/4 —
    scores |s|<=1.2 keep the cubic within fp8 noise).  This only pays
    with sp PSUM bufs=3 / po bufs=1 + a fast po->SBUF copy: with 2 sp
    buffers the pipeline runs at the slowest consumer's pace and the
    offload stretches the stream instead of shortening it.
Engine budget: ACT ~116us, PE ~110us, DVE ~100us, Pool ~20us; sim
161us, measured body ~105-160us (noisy probe) vs 242us baseline.
"""

import sys

for _p in ("/opt/trn_rl_repo", "/root/.axon_site/_ro/trn_rl_repo"):
    if _p not in sys.path:
        sys.path.insert(0, _p)

import ml_dtypes
import numpy as np

import concourse.mybir as mybir
from concourse import bacc
from concourse.bass import ds, ts
from concourse.tile import TileContext

import concourse.dve_ops as _dve_ops
from concourse.dve_spec import (
    C0 as _C0, C1 as _C1, C2 as _C2, One as _One,
    Spec as _Spec, Src0 as _S0, Src1 as _S1, lower as _dve_lower,
)
from concourse.dve_uop import DveOpSpec as _DveOpSpec


def _register_exp3():
    """Register (at runtime, so kernel.py stays self-contained) a custom DVE
    op computing  out = (1 + x*(c1 + x*(c2 + x*c3))) * y  — a cubic Horner
    exp() of the (1/16-scaled) score times a precomputed exp(adj)/4 factor.
    Scores |s| <= ~1.2 so the cubic is accurate to ~2% at the extreme tail,
    below the fp8 E quantization noise.  7 ALU stages, within the 8 limit."""
    name = "EXP3_MUL_ANT"
    for op in _dve_ops.OPS:
        if op.name == name:
            return op
    spec = _Spec(
        body=(((_S0 * _C0 + _C1) * _S0 + _C2) * _S0 + _One) * _S1,
        reference=lambda in0, in1, s0, s1, imm2: (
            (((in0 * s0 + s1) * in0 + imm2) * in0 + 1.0) * in1),
    )
    row = _dve_ops._CUSTOM_DVE_ROW_BASE + len(_dve_ops.OPS)
    assert row < 0x20
    shas = {}
    for ver in ("v3", "v4"):
        shas[ver] = _DveOpSpec(name=name, opcode=row,
                               uops=_dve_lower(spec, ver=ver),
                               rd1_en=True).sha(ver)
    op = _dve_ops.DveOp(name, spec, subdim=False, uops_sha=shas)
    _dve_ops.OPS.append(op)
    _dve_ops._SUB_OPCODE_FOR_NAME[name] = row
    _dve_ops.CUSTOM_DVE_SPECS[name] = spec
    return op


_EXP3 = _register_exp3()
_E3_C3, _E3_C2, _E3_C1 = 1.0 / 24576.0, 1.0 / 512.0, 1.0 / 16.0


def _dve_kts(h):
    """Key tiles whose exp runs on DVE (cubic poly).  Heads 0-1: few (the
    DVE is busy with projection copies then); later heads: 5 of 16."""
    if h == 0:
        return (15,)
    if h == 1:
        return (15,)
    return (3, 6, 9, 12, 15) if h < 5 else (4, 7, 11, 14)

B, N, C, H = 4, 2048, 256, 8
HD = C // H          # 32
NQ = N // 2          # 1024 query rows per core
SCALE = 1.0 / np.sqrt(HD)
FP32 = mybir.dt.float32
BF16 = mybir.dt.bfloat16
F8 = mybir.dt.float8e4
P = 128
BF16NP = ml_dtypes.bfloat16
F8NP = ml_dtypes.float8_e4m3

QS, KS = 8.0, 2.0            # fp8 pre-scales folded into host-side q/k weights
SSC = 1.0 / (QS * KS)        # exp() scale param undoing them
EBIAS = float(-2.0 * np.log(2.0))   # E *= 1/4: keeps exp under fp8e4 max
ADJ_C_STEP = 8.0             # c-part quantization step of adj*16
VW = 64                      # vF8 head stride (DoubleRow needs 64-col stationary tiles)

_CACHED = {}

PAIRS = ((0, 2), (1, 3), (4, 6), (5, 7))
DR = mybir.MatmulPerfMode.DoubleRow


def build_kernel(repeat=1):
    nc = bacc.Bacc("TRN2", target_bir_lowering=False)
    xt_ext = nc.declare_dram_parameter("xt", [C, NQ], BF16, isOutput=False)
    xf8_ext = nc.declare_dram_parameter("xf8", [P, 2, N], F8, isOutput=False)
    adjcr_ext = nc.declare_dram_parameter("adjcr", [P, 16, 2, 2, 512], F8, isOutput=False)
    eadj_ext = nc.declare_dram_parameter("eadj", [P, 16, 2, 512], BF16, isOutput=False)
    i2_ext = nc.declare_dram_parameter("i2", [P, 2, P], F8, isOutput=False)
    wf8_ext = nc.declare_dram_parameter("qkv_wf8", [P, 2, 3 * C], F8, isOutput=False)
    owt_ext = nc.declare_dram_parameter("out_wt", [C, C], BF16, isOutput=False)
    outb_ext = nc.declare_dram_parameter("out_b", [P, C], FP32, isOutput=False)
    out_ext = nc.declare_dram_parameter("out", [NQ, C], FP32, isOutput=True)

    with TileContext(nc) as tc:
        with (
            tc.tile_pool(name="const", bufs=1) as constp,
            tc.tile_pool(name="persist", bufs=1) as persist,
            tc.tile_pool(name="work", bufs=2) as work,
            tc.tile_pool(name="sp_pool", bufs=3, space="PSUM") as spp,
            tc.tile_pool(name="po_pool", bufs=1, space="PSUM") as pop,
        ):
            outb_bc = constp.tile([P, C], FP32)
            nc.gpsimd.dma_start(outb_bc[:], outb_ext[:, :])
            i2t = constp.tile([P, 2, P], F8)
            nc.gpsimd.dma_start(i2t[:], i2_ext[:, :, :])
            ebias = constp.tile([P, 1], FP32)
            nc.vector.memset(ebias[:], EBIAS)
            # k tiles are zero-padded ONCE here (one-time init: the zero
            # rows are never overwritten; per-body proj copies only touch
            # each head's own 32 rows).  Engines can't span >32 partitions
            # from a non-zero base, so zero quad by quad.
            kH8 = [persist.tile([P, N], F8, tag=f"kH8{i}", name=f"kH8{i}")
                   for i in range(H)]
            U32 = mybir.dt.uint32
            # all on DVE: ACT must stay free for the q/k group-0 copies
            # that gate the first exp (tile-level WAW ordering puts these
            # memzeros ahead of the k copies on whichever engine runs them)
            for hh in range(H):
                for r in range(0, P, 32):
                    if r != 32 * (hh % 4):
                        nc.vector.memzero(kH8[hh][ds(r, 32), :].bitcast(U32))
            for _ in range(repeat):
                _body(nc, tc, persist, work, spp, pop, outb_bc, i2t, ebias,
                      kH8,
                      xt_ext, xf8_ext, adjcr_ext, eadj_ext, wf8_ext, owt_ext,
                      out_ext)

    nc.compile()
    return nc


def _body(nc, tc, persist, work, spp, pop, outb_bc, i2t, ebias, kH8,
          xt_ext, xf8_ext, adjcr_ext, eadj_ext, wf8_ext, owt_ext, out_ext):
    AF = mybir.ActivationFunctionType
    ALU = mybir.AluOpType

    # ---------------- persistent SBUF tensors ----------------
    xT = [persist.tile([P, NQ], BF16, tag=f"xT{i}", name=f"xT{i}") for i in range(2)]
    xF8 = persist.tile([P, 2, N], F8, tag="xF8", name="xF8")
    wF8 = persist.tile([P, 2, 3 * C], F8, tag="wF8", name="wF8")
    owT = [persist.tile([P, C], BF16, tag=f"owT{i}", name=f"owT{i}") for i in range(2)]
    # q: 2 packed stacks [P, NQ] (head h at rows 32*(h%4) of stack h//4).
    # k: one zero-padded [P, N] tile per head — rows 32*(h%4)..+32 hold k_h,
    # the rest are zero, so score matmuls contract a full 128-row stationary
    # tile (measured: 32-row stationary tiles run ~2.3x slower than 128-row).
    qS8 = [persist.tile([P, NQ], F8, tag=f"qS8{i}", name=f"qS8{i}")
           for i in range(2)]
    vF8 = persist.tile([P, 16, H, VW], F8, tag="vF8")
    adjCR = persist.tile([P, 16, 2, 2, 512], F8, tag="adjCR")
    eadjT = persist.tile([P, 16, 2, 512], BF16, tag="eadjT")
    attT = [persist.tile([P, NQ], BF16, tag=f"attT{i}", name=f"attT{i}")
            for i in range(2)]

    # ---------------- loads (already transposed/scaled on host) -----------
    for s in range(2):
        nc.sync.dma_start(xF8[:, s, :], xf8_ext[:, s, :])
    nc.sync.dma_start(wF8[:], wf8_ext[:, :, :])
    for i in range(16):
        nc.gpsimd.dma_start(adjCR[:, i, :, :, :], adjcr_ext[:, i, :, :, :])
    for j in range(2):
        nc.sync.dma_start(xT[j][:], xt_ext[ds(j * P, P), :])
        nc.sync.dma_start(owT[j][:], owt_ext[ds(j * P, P), :])
    for i in range(16):
        nc.sync.dma_start(eadjT[:, i, :, :], eadj_ext[:, i, :, :])

    # -------- QKV projections (fp8 DoubleRow PE, fp8 outputs) -------------
    # x/w ship from the host in [P, 2, *] fp8 (channel c = 128*slot + p),
    # so one DR call contracts all 256 channels.  Weights carry a x16
    # pre-scale (keeps fp8 weights out of the subnormal range); the
    # PSUM->fp8 copies divide it back out.
    # Scheduling: head 0/1 only needs q/k group 0 — its copies go on the
    # (pre-stream idle) ACT engine so the exp stream starts ~6us in.  The
    # v copies run on DVE next (attnv consumes them progressively), and
    # groups 1-3 are emitted mid-attention (see the h==0 hook below) so
    # their copies hide under the exp stream instead of walling it off.
    # Projection PSUM comes from the po pool, which attention barely uses
    # early on; the score sp tiles stay free of proj WAR hazards.
    DSC = 1.0 / 16.0

    def emit_qk_unit(g, off, nch, copy_eng, psum_tag="sp"):
        # one 64-row matmul+copy for heads (2g, 2g+1); off 0 = q, C = k
        pool_ = pop if psum_tag == "po" else spp
        pp = pool_.tile([P, NQ], FP32, tag=psum_tag, name="pp",
                        bufs=1 if psum_tag == "po" else 3)[:, :512]
        nc.tensor.matmul(pp[ds(0, 64), :],
                         wF8[:, :, ds(off + 64 * g, 64)],
                         xF8[:, :, ts(nch, 512)],
                         start=True, stop=True, perf_mode=DR,
                         skip_group_check=True)
        cp = (nc.scalar.mul if copy_eng == "act"
              else nc.vector.tensor_scalar_mul)
        if off == 0:
            cp(qS8[g // 2][ds(64 * (g % 2), 64), ts(nch, 512)],
               pp[ds(0, 64), :], DSC)
        else:
            for i in range(2):
                hh = 2 * g + i
                cp(kH8[hh][ds(32 * (hh % 4), 32), ts(nch, 512)],
                   pp[ds(32 * i, 32), :], DSC)

    def emit_qk_group(g, copy_eng, alt=False):
        i = 0
        for off, nchs in ((0, 2), (C, 4)):
            for nch in range(nchs):
                emit_qk_unit(g, off, nch, copy_eng,
                             "po" if (alt and i % 2) else "sp")
                i += 1

    def emit_v(kt):
        pv = spp.tile([P, NQ], FP32, tag="sp", name="pv")[:, :C]
        nc.tensor.matmul(pv[:, :C], xF8[:, :, ts(kt, P)],
                         wF8[:, :, ds(2 * C, C)],
                         start=True, stop=True, perf_mode=DR,
                         skip_group_check=True)
        nc.vector.tensor_scalar_mul(
            vF8[:, kt, :, 0:HD],
            pv[:, :C].rearrange("p (h d) -> p h d", h=H), DSC)

    emit_qk_group(0, "act", alt=True)
    emit_qk_group(1, "dve", alt=True)  # heads 0-3 share a q stack: pre-stream
    # v: [key_tile, head, hd] fp8 with ones column scaled by 10 (folds 0.1);
    # cols 33..63 are junk (never read: po rows 33..63 are dead)
    nc.vector.memset(vF8[:, :, :, HD], 10.0)
    for kt in range(4):
        emit_v(kt)       # kt 4..15 trickle through head 0's stream below

    # ---------------- attention: one head at a time ----------------
    # (DoubleRow matmuls require output tile_position col 0, so each head's
    # attnv accumulates into rows 0..63 of its own rotating po tile.)
    # attnv emission lags the scores/exp stream by 2 kt-pairs: the PE is
    # in-order, so an attnv right after its exp would stall the PE (and
    # starve ACT, the bottleneck) until the exp finishes.  With the lag the
    # PE always has ready work and ACT stays saturated.
    pos = {}

    def emit_attnv(ph, ptp, pE):
        if ptp == 0:
            pos[ph] = pop.tile([P, NQ], FP32, tag="po", name="po", bufs=1)
        po = pos[ph]
        for c in range(2):
            nc.tensor.matmul(po[ds(0, 64), ds(512 * c, 512)],
                             vF8[:, ds(2 * ptp, 2), ph, ds(0, 64)],
                             pE[:, c, :, :],
                             start=(ptp == 0), stop=(ptp == 7), perf_mode=DR,
                             skip_group_check=True)
        if ptp == 7:
            emit_norm(ph, po)

    def emit_norm(ph, po):
        # copy po out fast: with po bufs=1 the next head's attnv waits on
        # this copy, so it must be the ONLY reader of po
        por = work.tile([P, NQ], FP32, tag="por", name="por")
        nc.vector.tensor_copy(por[:], po[:])
        # reciprocal on a DMA-spread [32,32] layout (DVE recip cost scales
        # with free size; the spread form is ~16x cheaper than [1,1024])
        dsp = work.tile([32, 32], FP32, tag="dsp", name="dsp")
        nc.sync.dma_start(dsp[:],
                          por[ds(HD, 1), :].rearrange("o (p j) -> o p j", p=32))
        rc = work.tile([32, 32], FP32, tag="rc", name="rc")
        nc.vector.reciprocal(rc[:], dsp[:])
        rr = work.tile([1, NQ], FP32, tag="rr", name="rr")
        nc.sync.dma_start(rr[:].rearrange("o (p j) -> o p j", p=32), rc[:])
        bc = work.tile([HD, NQ], FP32, tag="bc", name="bc")
        nc.sync.dma_start(bc[:], rr[ds(0, 1), None, :].to_broadcast((1, HD, NQ)))
        home, chunk = 32 * (ph % 4), ph // 4
        nc.vector.tensor_tensor(attT[chunk][ds(home, HD), :],
                                por[ds(0, HD), :], bc[:], ALU.mult)
        if ph % 4 == 3:
            # this attT chunk is complete: fold the residual add in now so
            # the out-proj tail only waits on the final head's normalize
            cc = ph // 4
            nc.vector.tensor_tensor(attT[cc][:], attT[cc][:], xT[cc][:],
                                    ALU.add)

    late_qk = [(g, off, nch) for g in (2, 3)
               for off, nchs in ((0, 2), (C, 4)) for nch in range(nchs)]
    pend = []
    for h in range(H):
        for tp in range(8):
            E2 = work.tile([P, 2, 2, 512], F8, tag="E2", name="E2", bufs=5)
            for half in range(2):
                kt = 2 * tp + half
                dve_tile = kt in _dve_kts(h)
                sp = spp.tile([P, NQ], FP32, tag="sp", name="sp")
                for c in range(2):
                    if not dve_tile:
                        # adj preload: DR identity matmul, slot0=c slot1=r
                        nc.tensor.matmul(sp[:, ds(512 * c, 512)],
                                         i2t[:, :, :],
                                         adjCR[:, kt, c, :, :],
                                         start=True, stop=False, perf_mode=DR,
                                         skip_group_check=True)
                    # scores: zero-padded 128-row stationary k, plain fp8
                    nc.tensor.matmul(sp[:, ds(512 * c, 512)],
                                     kH8[h][:, ts(kt, P)],
                                     qS8[h // 4][:, ds(512 * c, 512)],
                                     start=dve_tile, stop=True,
                                     skip_group_check=True)
                if dve_tile:
                    # offload exp to DVE: cubic-poly exp(s) * exp(adj)/4
                    # (raw scores only — adj never touches PSUM here)
                    for c in range(2):
                        nc.vector._custom_dve(
                            _EXP3, out=E2[:, c, half, :],
                            in0=sp[:, ds(512 * c, 512)],
                            in1=eadjT[:, kt, c, :],
                            s0=_E3_C3, s1=_E3_C2, imm2=_E3_C1)
                else:
                    nc.scalar.activation(
                        E2[:, :, half, :],
                        sp[:].rearrange("p (c j) -> p c j", c=2),
                        AF.Exp, bias=ebias[:], scale=SSC)
            pend.append((h, tp, E2))
            while len(pend) > (3 if h < H - 1 else 1):
                emit_attnv(*pend.pop(0))
            if h == 0 and tp < 6:
                # v proj kt 4..15 trickles through head 0's sp rotation
                emit_v(2 * tp + 4)
                emit_v(2 * tp + 5)
            if h in (1, 2) and late_qk:
                # groups 2-3 (needed from head 4) trickle through heads 1-2,
                # one unit per kt-pair so the DVE never backs up the sp
                # rotation (2/tp overloaded DVE at the head-1/2 boundary)
                emit_qk_unit(*late_qk.pop(0), "dve")
    while pend:
        emit_attnv(*pend.pop(0))

    # ---------------- out_proj (residual folded in emit_norm) -------------
    for rt in range(8):
        pf = spp.tile([P, NQ], FP32, tag="sp", name="pf")[:, :C]
        for cc in range(2):
            nc.tensor.matmul(pf[:, :C], attT[cc][:, ts(rt, P)], owT[cc][:],
                             start=(cc == 0), stop=(cc == 1))
        osb = work.tile([P, C], FP32, tag="osb", name="osb", bufs=6)
        nc.vector.tensor_tensor(osb[:], pf[:, :C], outb_bc[:], ALU.add)
        nc.sync.dma_start(out_ext[ds(rt * P, P), :], osb[:])


def _run(nc, in_maps):
    from concourse.bass_utils import run_bass_kernel_spmd
    res = run_bass_kernel_spmd(nc, in_maps, core_ids=list(range(8)))
    return res.results


def make_in_maps(x, adj, qkv_w, out_w, out_b):
    x = np.asarray(x, np.float32)
    adj = np.asarray(adj, np.float32)
    w = np.asarray(qkv_w, np.float32).copy()
    w[:C] *= SCALE * QS * 16.0           # fold 1/sqrt(HD) + fp8 q scale
    w[C:2 * C] *= KS * 16.0              # fp8 k scale
    w[2 * C:] *= 16.0
    # [C, 3C] -> [128, 2, 3C] fp8 with channel c = 128*slot + p
    wf8 = np.ascontiguousarray(
        w.T.reshape(2, P, 3 * C).transpose(1, 0, 2)).astype(F8NP)
    owt = np.ascontiguousarray(np.asarray(out_w, np.float32).T).astype(BF16NP)
    outb = np.ascontiguousarray(
        np.broadcast_to(np.asarray(out_b, np.float32), (P, C)))
    i2 = np.zeros((P, 2, P), F8NP)
    for p in range(P):
        i2[p, :, p] = F8NP(1.0)
    in_maps = []
    for c in range(8):
        b, half = divmod(c, 2)
        xb = np.roll(x[b], -half * NQ, axis=0)
        xt = np.ascontiguousarray(xb.T[:, :NQ]).astype(BF16NP)  # [C, NQ]
        xf8 = np.ascontiguousarray(
            xb.T.reshape(2, P, N).transpose(1, 0, 2)).astype(F8NP)
        aj = np.roll(adj[half * NQ:(half + 1) * NQ, :], -half * NQ, axis=1)
        a16 = aj.T * (QS * KS)                                  # [N, NQ]
        ac = np.round(a16 / ADJ_C_STEP) * ADJ_C_STEP
        ar = a16 - ac
        # [N, NQ] -> [16, P, NQ] -> [P, 16, NQ]; stack (c, r) on a new axis
        acT = ac.reshape(16, P, NQ).transpose(1, 0, 2)
        arT = ar.reshape(16, P, NQ).transpose(1, 0, 2)
        # [P,16,2slot,NQ] -> [P,16,2chunk,2slot,512] (contiguous DR chunks)
        st = np.stack([acT, arT], axis=2).reshape(P, 16, 2, 2, 512)
        adjcr = np.ascontiguousarray(st.transpose(0, 1, 3, 2, 4)).astype(F8NP)
        # exp(adj)/4 for the DVE cubic-poly exp path (1/4 = the EBIAS fold)
        ea = (np.exp(aj.T) * 0.25).reshape(16, P, NQ).transpose(1, 0, 2)
        eadj = np.ascontiguousarray(ea.reshape(P, 16, 2, 512)).astype(BF16NP)
        in_maps.append({
            "xt": xt, "xf8": xf8, "adjcr": adjcr, "eadj": eadj, "i2": i2,
            "qkv_wf8": wf8, "out_wt": owt, "out_b": outb,
        })
    return in_maps


def kernel(x, x0, adj, qkv_w, out_w, out_b, pos_w, pos_b):
    """Full-input, full-output entry point.  x0/pos_w/pos_b are dead in the
    reference computation and are ignored."""
    if "nc" not in _CACHED:
        _CACHED["nc"] = build_kernel(repeat=1)
    nc = _CACHED["nc"]
    in_maps = make_in_maps(x, adj, qkv_w, out_w, out_b)
    results = _run(nc, in_maps)
    out = np.empty((B, N, C), np.float32)
    for c in range(8):
        b, half = divmod(c, 2)
        out[b, half * NQ:(half + 1) * NQ, :] = results[c]["out"]
    return out
